# revision 1
# baseline (speedup 1.0000x reference)
"""Trainium2 Bass kernel for nn_ContextQueryAttentionLayer.

Math: with B,N,M,D = 32,1024,256,128 the reference's gather index collapses:
  idx[i,j] = (i*M + j) % N = 256*(i%4) + j          (since M=256, N=1024)
so the similarity matrix S (b,n,m) has only 4 distinct rows per batch,
S[b,i,:] = t[b, i%4, :] with t (4,256):
  t[r,j] = q_j.w_q + c_{256r+j}.w_c + sum_d q_{j,d} w_m_d c_{256r+j,d}
Both softmaxes, c2q, sm (reduces to a 4x4 matrix per batch) and q2c then
collapse to rank-4-per-batch quantities, leaving a DMA-bound kernel:
  out[b,n] = [ctx_n, C2Q[n%4], ctx_n*C2Q[n%4], ctx_n*Q2C[n%4]]

Sharding: data-parallel over batch, 4 batches per core on 8 cores.
On-core layout: rows n=128k+p -> partition p (so n%4 == p%4). Query-only
prep (qwc, s_q) and the context column-sum tree are batched across all 4
resident batches; the per-batch t-columns pipeline POOL multiplies into DVE
reduces, one PE transpose moves t into an (8,128) softmax domain where all
scalings are per-partition, and the batch tail (products of context with
the broadcast C2Q/Q2C rows) is written by split DMA streams so no on-chip
assembly copies are needed.
"""

import numpy as np

B, N, M, D = 32, 1024, 256, 128
NCORES = 8
BPC = B // NCORES  # batches per core

_prog = None

# packed constant layout: name -> (partitions, col_start, col_len)
_CST_COLS = {
    "ident": (128, 0, 128),
    "wmb": (128, 128, 128),
    "wcb": (128, 256, 128),
    "wqb": (128, 384, 128),
    "b4": (4, 512, 128),
    "i16": (16, 640, 16),
    "pairsel": (16, 656, 8),
    "pairselT": (8, 664, 16),
    "hsel": (16, 680, 4),
    "rsel": (128, 700, 4),
}
_CST_W = 704


def _build_program():
    import concourse.bacc as bacc
    import concourse.mybir as mybir
    from concourse.tile import TileContext

    fp32 = mybir.dt.float32
    nc = bacc.Bacc("TRN2", target_bir_lowering=False, name="cqattn")

    ctx_d = nc.dram_tensor("ctx", [BPC, N, D], fp32, kind="ExternalInput")
    qry_d = nc.dram_tensor("qry", [BPC, M, D], fp32, kind="ExternalInput")
    cstp_d = nc.dram_tensor("cstp", [128, _CST_W], fp32, kind="ExternalInput")
    out_d = nc.dram_tensor("out", [BPC, N, 4 * D], fp32, kind="ExternalOutput")

    Exp = mybir.ActivationFunctionType.Exp
    Copy = mybir.ActivationFunctionType.Copy
    add = mybir.AluOpType.add
    X = mybir.AxisListType.X

    with TileContext(nc) as tc:
        with (
            tc.tile_pool(name="consts", bufs=1) as consts,
            tc.tile_pool(name="io", bufs=1) as io,
            tc.tile_pool(name="work", bufs=2) as work,
            tc.tile_pool(name="small", bufs=2) as small,
            tc.tile_pool(name="outp", bufs=2) as outp,
            tc.tile_pool(name="ps_tr", bufs=2, space="PSUM") as ps_tr,
            tc.tile_pool(name="ps_sm", bufs=1, space="PSUM") as ps_sm,
            tc.tile_pool(name="ps_mm", bufs=2, space="PSUM") as ps_mm,
            tc.tile_pool(name="ps_cs", bufs=1, space="PSUM") as ps_cs,
            tc.tile_pool(name="ps_rep", bufs=2, space="PSUM") as ps_rep,
        ):
            cstp = consts.tile([128, _CST_W], fp32, tag="cstp", name="cstp")
            nc.sync.dma_start(out=cstp, in_=cstp_d[...])
            cst = {
                n: cstp[:p, c0 : c0 + cl] for n, (p, c0, cl) in _CST_COLS.items()
            }

            # ---- all loads up front: rows n=128k+p -> partition p, block k
            ctx_mega = io.tile([128, BPC, 8, 128], fp32, tag="ctx", name="ctx_mega")
            qry_mega = io.tile([128, BPC, 2, 128], fp32, tag="qry", name="qry_mega")
            for b in range(BPC):
                nc.sync.dma_start(
                    out=ctx_mega[:, b],
                    in_=ctx_d[b].rearrange("(k p) d -> p k d", p=128),
                )
                nc.sync.dma_start(
                    out=qry_mega[:, b],
                    in_=qry_d[b].rearrange("(h p) d -> p h d", p=128),
                )
                # output stream a: raw context columns (no compute needed)
                nc.scalar.dma_start(
                    out=out_d[b][:, 0:128].rearrange("(k p) c -> p k c", p=128),
                    in_=ctx_mega[:, b],
                )

            # ---- batched query prep: qwcT = qry*w_m + w_c, sq = qry . w_q
            qwcT = work.tile([128, BPC, 2, 128], fp32, tag="qwcT")
            nc.vector.tensor_mul(
                qwcT,
                qry_mega,
                cst["wmb"]
                .rearrange("p (u v d) -> p u v d", u=1, v=1)
                .to_broadcast([128, BPC, 2, 128]),
            )
            nc.vector.tensor_add(
                qwcT,
                qwcT,
                cst["wcb"]
                .rearrange("p (u v d) -> p u v d", u=1, v=1)
                .to_broadcast([128, BPC, 2, 128]),
            )
            sq_tmp = work.tile([128, BPC, 2, 128], fp32, tag="sq_tmp")
            nc.vector.tensor_mul(
                sq_tmp,
                qry_mega,
                cst["wqb"]
                .rearrange("p (u v d) -> p u v d", u=1, v=1)
                .to_broadcast([128, BPC, 2, 128]),
            )
            sq_col = small.tile([128, BPC, 2], fp32, tag="sq_col")
            nc.vector.tensor_reduce(out=sq_col, in_=sq_tmp, axis=X, op=add)

            # ---- batched CS tree: csum[p,b,d] = sum_k ctx[b,128k+p,d]
            tmp4 = work.tile([128, BPC, 4, 128], fp32, tag="tmp4")
            nc.vector.tensor_add(
                tmp4, ctx_mega[:, :, 0:4, :], ctx_mega[:, :, 4:8, :]
            )
            tmp2 = work.tile([128, BPC, 2, 128], fp32, tag="tmp2")
            nc.gpsimd.tensor_add(tmp2, tmp4[:, :, 0:2, :], tmp4[:, :, 2:4, :])
            csum = work.tile([128, BPC, 128], fp32, tag="csum")
            nc.gpsimd.tensor_add(csum, tmp2[:, :, 0, :], tmp2[:, :, 1, :])
            cs_ps = ps_cs.tile([4, BPC, 128], fp32, tag="cs")
            nc.tensor.matmul(cs_ps, cst["rsel"], csum, start=True, stop=True)
            cs = small.tile([4, BPC, 128], fp32, tag="cs")
            nc.scalar.copy(out=cs, in_=cs_ps)

            for b in range(BPC):
                ctx_b = ctx_mega[:, b]
                qry_b = qry_mega[:, b]

                # ---- t columns: t_sb[p, 2r+h] = t[r, 128h+p]
                # POOL multiplies, DVE reduces (pipelined per h)
                t_sb = small.tile([128, 8], fp32, tag="t_sb")
                ctx_v = ctx_b.rearrange("p (r h) d -> p h r d", h=2)
                t_v = t_sb[:, :].rearrange("p (r h) -> p h r", h=2)
                for h in range(2):
                    g_tmp = work.tile([128, 4, 128], fp32, tag="g_tmp")
                    nc.gpsimd.tensor_mul(
                        g_tmp,
                        ctx_v[:, h],
                        qwcT[:, b, h, :]
                        .rearrange("p (u d) -> p u d", u=1)
                        .to_broadcast([128, 4, 128]),
                    )
                    nc.vector.tensor_reduce(
                        out=t_v[:, h], in_=g_tmp, axis=X, op=add
                    )
                nc.vector.tensor_add(
                    t_sb[:, :].rearrange("p (r h) -> p r h", h=2),
                    t_sb[:, :].rearrange("p (r h) -> p r h", h=2),
                    sq_col[:, b, :]
                    .rearrange("p (u h) -> p u h", u=1)
                    .to_broadcast([128, 4, 2]),
                )

                # ---- transpose to (8,128): row q = 2r+h, free p
                t8_ps = ps_tr.tile([8, 128], fp32, tag="tr")
                nc.tensor.transpose(t8_ps, t_sb, cst["ident"])

                # ---- softmaxes (no max-shift: |t| < ~8)
                e8 = small.tile([8, 128], fp32, tag="e8")
                rowsumc = small.tile([8, 1], fp32, tag="rowsumc")
                nc.scalar.activation(out=e8, in_=t8_ps, func=Exp, accum_out=rowsumc)
                # soft_c scale: rowsums per r via pairsel, reciprocal, broadcast
                pairs_ps = ps_sm.tile([4, 1], fp32, tag="sm")
                nc.tensor.matmul(
                    pairs_ps, cst["pairsel"][:8, :4], rowsumc, start=True, stop=True
                )
                rec4 = small.tile([4, 1], fp32, tag="rec4")
                nc.vector.reciprocal(out=rec4, in_=pairs_ps)
                rec8_ps = ps_sm.tile([8, 1], fp32, tag="sm")
                nc.tensor.matmul(
                    rec8_ps, cst["pairselT"][:4, :8], rec4, start=True, stop=True
                )
                rec8 = small.tile([8, 1], fp32, tag="rec8")
                nc.vector.tensor_copy(out=rec8, in_=rec8_ps)
                sc8 = small.tile([8, 128], fp32, tag="sc8")
                nc.scalar.activation(out=sc8, in_=e8, func=Copy, scale=rec8)
                # soft_q denominators: u2[h,p] = sum_r e8[2r+h,p]
                u2_ps = ps_sm.tile([2, 128], fp32, tag="sm")
                nc.tensor.matmul(
                    u2_ps, cst["hsel"][:8, :2], e8, start=True, stop=True
                )
                u2 = small.tile([2, 128], fp32, tag="u2")
                nc.scalar.copy(out=u2, in_=u2_ps)

                # ---- transposed-domain soft rows (128, 8): col q = 2r+h
                scT_ps = ps_tr.tile([128, 8], fp32, tag="tr")
                nc.tensor.transpose(scT_ps, sc8, cst["i16"][:8, :8])
                scT2 = small.tile([128, 8], fp32, tag="scT")
                nc.vector.tensor_copy(out=scT2, in_=scT_ps)
                scT = scT2[:, :].rearrange("p (r h) -> p r h", r=4)
                eT_ps = ps_tr.tile([128, 8], fp32, tag="tr")
                nc.tensor.transpose(eT_ps, e8, cst["i16"][:8, :8])
                u2T_ps = ps_tr.tile([128, 2], fp32, tag="tr")
                nc.tensor.transpose(u2T_ps, u2, cst["i16"][:2, :2])
                recu = small.tile([128, 2], fp32, tag="recu")
                nc.vector.reciprocal(out=recu, in_=u2T_ps)
                sqT2 = small.tile([128, 8], fp32, tag="sqT")
                nc.vector.tensor_mul(
                    sqT2[:, :].rearrange("p (r h) -> p r h", r=4),
                    eT_ps[:, :].rearrange("p (r h) -> p r h", r=4),
                    recu[:, :]
                    .rearrange("p (u h) -> p u h", u=1)
                    .to_broadcast([128, 4, 2]),
                )
                sqT = sqT2[:, :].rearrange("p (r h) -> p r h", r=4)

                # ---- SM4T[r',r] = sum_j sq[r',j] sc[r,j], scaled by 1/256
                sm4t_ps = ps_mm.tile([4, 4], fp32, tag="mm")
                for h in range(2):
                    nc.tensor.matmul(
                        sm4t_ps, sqT[:, :, h], scT[:, :, h],
                        start=(h == 0), stop=(h == 1),
                    )
                sm4t = small.tile([4, 4], fp32, tag="sm4t")
                nc.vector.tensor_scalar_mul(sm4t, sm4t_ps, 1.0 / 256.0)

                # ---- C2Q[r,d] = sum_j sc[r,j] qry[j,d]
                c2q_ps = ps_mm.tile([4, 128], fp32, tag="mm")
                for h in range(2):
                    nc.tensor.matmul(
                        c2q_ps, scT[:, :, h], qry_b[:, h, :],
                        start=(h == 0), stop=(h == 1),
                    )
                c2q = small.tile([4, 128], fp32, tag="c2q")
                nc.scalar.copy(out=c2q, in_=c2q_ps)

                # ---- Q2C[r,d] = sum_{r'} SM4[r,r'] CS[r',d]
                q2c_ps = ps_mm.tile([4, 128], fp32, tag="mm")
                nc.tensor.matmul(q2c_ps, sm4t, cs[:, b, :], start=True, stop=True)
                q2c = small.tile([4, 128], fp32, tag="q2c")
                nc.scalar.copy(out=q2c, in_=q2c_ps)

                # ---- broadcast rows r -> 128 partitions (p%4 pattern)
                repc_ps = ps_rep.tile([128, 128], fp32, tag="rep")
                nc.tensor.matmul(repc_ps, cst["b4"], c2q, start=True, stop=True)
                repc = small.tile([128, 128], fp32, tag="repc")
                nc.scalar.copy(out=repc, in_=repc_ps)
                repq_ps = ps_rep.tile([128, 128], fp32, tag="rep")
                nc.tensor.matmul(repq_ps, cst["b4"], q2c, start=True, stop=True)
                repq = small.tile([128, 128], fp32, tag="repq")
                nc.scalar.copy(out=repq, in_=repq_ps)

                # ---- output streams b (broadcast C2Q cols) and c/d (products)
                nc.scalar.dma_start(
                    out=out_d[b][:, 128:256].rearrange("(k p) c -> p k c", p=128),
                    in_=repc[:, :]
                    .rearrange("p (u d) -> p u d", u=1)
                    .to_broadcast([128, 8, 128]),
                )
                out_sb = outp.tile([128, 8, 2, 128], fp32, tag="out")
                nc.vector.tensor_mul(
                    out_sb[:, :, 0, :],
                    ctx_b,
                    repc[:, :]
                    .rearrange("p (u d) -> p u d", u=1)
                    .to_broadcast([128, 8, 128]),
                )
                nc.sync.dma_start(
                    out=out_d[b][:, 256:384].rearrange("(k p) c -> p k c", p=128),
                    in_=out_sb[:, :, 0, :],
                )
                eng = nc.vector if b == BPC - 1 else nc.gpsimd
                eng.tensor_mul(
                    out_sb[:, :, 1, :],
                    ctx_b,
                    repq[:, :]
                    .rearrange("p (u d) -> p u d", u=1)
                    .to_broadcast([128, 8, 128]),
                )
                nc.sync.dma_start(
                    out=out_d[b][:, 384:512].rearrange("(k p) c -> p k c", p=128),
                    in_=out_sb[:, :, 1, :],
                )
    nc.compile()
    return nc


def _get_program():
    global _prog
    if _prog is None:
        _prog = _build_program()
    return _prog


def _make_const_inputs(w):
    w = np.ascontiguousarray(w, dtype=np.float32)
    w_q, w_c, w_m = w[:D, 0], w[D : 2 * D, 0], w[2 * D :, 0]
    p = np.arange(128)
    q = np.arange(16)
    pairsel = (q[:, None] // 2 == np.arange(8)[None, :]).astype(np.float32)
    hsel = (
        2 * (q[:, None] // 8) + (q[:, None] % 2) == np.arange(4)[None, :]
    ).astype(np.float32)
    vals = {
        "ident": np.eye(128, dtype=np.float32),
        "i16": np.eye(16, dtype=np.float32),
        "wmb": np.broadcast_to(w_m[None, :], (128, 128)),
        "wcb": np.broadcast_to(w_c[None, :], (128, 128)),
        "wqb": np.broadcast_to(w_q[None, :], (128, 128)),
        "pairsel": pairsel,
        "pairselT": pairsel.T,
        "hsel": hsel,
        "rsel": (p[:, None] % 4 == np.arange(4)[None, :]).astype(np.float32),
        "b4": (np.arange(4)[:, None] == p[None, :] % 4).astype(np.float32),
    }
    packed = np.zeros((128, _CST_W), dtype=np.float32)
    for n, (parts, c0, cl) in _CST_COLS.items():
        packed[:parts, c0 : c0 + cl] = vals[n]
    return {"cstp": packed}


def _run(context, query, w, trace=False):
    from concourse.bass_utils import run_bass_kernel_spmd

    nc = _get_program()
    context = np.ascontiguousarray(context, dtype=np.float32)
    query = np.ascontiguousarray(query, dtype=np.float32)
    consts = _make_const_inputs(w)

    in_maps = []
    for c in range(NCORES):
        m = {
            "ctx": context[c * BPC : (c + 1) * BPC],
            "qry": query[c * BPC : (c + 1) * BPC],
        }
        m.update(consts)
        in_maps.append(m)

    res = run_bass_kernel_spmd(
        nc, in_maps, core_ids=list(range(NCORES)), trace=trace
    )
    out = np.concatenate([res.results[c]["out"] for c in range(NCORES)], axis=0)
    return out, res


def kernel(context, query, c_mask, q_mask, w):
    out, _ = _run(context, query, w, trace=False)
    return out



# revision 2
# speedup vs baseline: 1.1749x; 1.1749x over previous
"""Trainium2 Bass kernel for nn_ContextQueryAttentionLayer (v8).

Math: idx collapses, S[b,i,:] = t[b, i%4, :]; softmaxes/c2q/sm/q2c are
rank-4 per batch; kernel is HBM-bound (10.5 MB/core, ~30 us roofline).

v8: fully per-batch pipeline, including the loads.  Key lesson from the
v5-v7 traces: engines execute their instruction queues IN ORDER, so any
op emitted early but gated late (e.g. a later batch's ctx->row copy on
ACT) stalls every later op on that engine; and the scheduler orders each
engine's queue by simulated readiness, so batch 0's softmax must be
READY (ACT exp not queue-blocked) when DVE frees or later batches' big
t-ops get packed first.  v8 therefore interleaves qry(b)/ctx(b) loads
(simpler APs also cut HWDGE descriptor-generation latency at the head)
and emits each batch's FULL chain together:
  loads(b) -> qwc(b) -> t(b) -> softmax(b) -> projections(b) ->
  row assembly(b) -> one 2MB full-row DMA (2KB descriptors).
GpSimd takes s_q and the odd batches' product; ACT takes the raw-ctx
and C2Q-broadcast row columns; the PE accumulates cs.

Sharding: data-parallel over batch, 4 batches per core on 8 cores.
On-core layout: ctx rows n=128k+p -> partition p (n%4 == p%4); query
rows j=128h+p -> partition p.
"""

import numpy as np

B, N, M, D = 32, 1024, 256, 128
NCORES = 8
BPC = B // NCORES  # batches per core

_prog = None

# packed constant layout: name -> (partitions, col_start, col_len)
_CST_COLS = {
    "wmb": (128, 0, 128),
    "wcb": (128, 128, 128),
    "wqb": (128, 256, 128),
    "b4": (4, 384, 128),
    "rsel": (128, 512, 4),
    "ones1": (128, 516, 1),
}
_CST_W = 517


def _build_program():
    import concourse.bacc as bacc
    import concourse.mybir as mybir
    from concourse.tile import TileContext

    fp32 = mybir.dt.float32
    nc = bacc.Bacc("TRN2", target_bir_lowering=False, name="cqattn")

    ctx_d = nc.dram_tensor("ctx", [BPC, N, D], fp32, kind="ExternalInput")
    qry_d = nc.dram_tensor("qry", [BPC, M, D], fp32, kind="ExternalInput")
    cstp_d = nc.dram_tensor("cstp", [128, _CST_W], fp32, kind="ExternalInput")
    out_d = nc.dram_tensor("out", [BPC, N, 4 * D], fp32, kind="ExternalOutput")

    Exp = mybir.ActivationFunctionType.Exp
    add = mybir.AluOpType.add
    X = mybir.AxisListType.X

    with TileContext(nc) as tc:
        with (
            tc.tile_pool(name="consts", bufs=1) as consts,
            tc.tile_pool(name="io", bufs=1) as io,
            tc.tile_pool(name="one", bufs=1) as one,
            tc.tile_pool(name="gwork", bufs=2) as gwork,
            tc.tile_pool(name="pb", bufs=2) as pb,
            tc.tile_pool(name="outp", bufs=4) as outp,
            tc.tile_pool(name="ps_sm", bufs=2, space="PSUM") as ps_sm,
            tc.tile_pool(name="ps_cs", bufs=2, space="PSUM") as ps_cs,
            tc.tile_pool(name="ps_mm", bufs=2, space="PSUM") as ps_mm,
            tc.tile_pool(name="ps_rep", bufs=2, space="PSUM") as ps_rep,
        ):
            # consts first on the scalar ring (simple AP, lands ~2.5us)
            cstp = consts.tile([128, _CST_W], fp32, tag="cstp", name="cstp")
            nc.scalar.dma_start(out=cstp, in_=cstp_d[...])
            cst = {
                n: cstp[:p, c0 : c0 + cl] for n, (p, c0, cl) in _CST_COLS.items()
            }

            qry_mega = io.tile([128, BPC, 2, 128], fp32, tag="qry", name="qry_mega")
            ctx_mega = io.tile([128, BPC, 8, 128], fp32, tag="ctx", name="ctx_mega")

            # shared softmax-domain tiles (written per batch, disjoint slices)
            qwcT = one.tile([128, BPC, 2, 128], fp32, tag="qwcT")
            sq_tmp = one.tile([128, BPC, 2, 128], fp32, tag="sq_tmp")
            sq_col = one.tile([128, BPC, 2], fp32, tag="sq_col")
            t_all = one.tile([128, BPC, 8], fp32, tag="t_all")
            cs = one.tile([4, BPC, 128], fp32, tag="cs")
            eT = one.tile([128, BPC, 8], fp32, tag="eT")
            epair = one.tile([128, BPC, 4], fp32, tag="epair")
            u4 = one.tile([128, BPC, 4], fp32, tag="u4")
            uT = one.tile([128, BPC, 2], fp32, tag="uT")
            recu = one.tile([128, BPC, 2], fp32, tag="recu")
            sqT = one.tile([128, BPC, 8], fp32, tag="sqT")
            eT_v = eT[:, :, :].rearrange("p b (r h) -> p b r h", h=2)
            sqT_v = sqT[:, :, :].rearrange("p b (r h) -> p b r h", h=2)

            for b in range(BPC):
                s = slice(b, b + 1)
                # ---- loads for this batch (sync ring, simple APs)
                nc.sync.dma_start(
                    out=qry_mega[:, b],
                    in_=qry_d[b].rearrange("(h p) d -> p h d", p=128),
                )
                nc.sync.dma_start(
                    out=ctx_mega[:, b],
                    in_=ctx_d[b].rearrange("(k p) d -> p k d", p=128),
                )
                row = outp.tile([128, 8, 512], fp32, tag="row")
                nc.scalar.copy(out=row[:, :, 0:128], in_=ctx_mega[:, b])

                # ---- qwc(b) on DVE, s_q(b) on GpSimd(+DVE reduce)
                nc.vector.tensor_mul(
                    qwcT[:, b],
                    qry_mega[:, b],
                    cst["wmb"]
                    .rearrange("p (u d) -> p u d", u=1)
                    .to_broadcast([128, 2, 128]),
                )
                nc.vector.tensor_add(
                    qwcT[:, b],
                    qwcT[:, b],
                    cst["wcb"]
                    .rearrange("p (u d) -> p u d", u=1)
                    .to_broadcast([128, 2, 128]),
                )
                nc.gpsimd.tensor_mul(
                    sq_tmp[:, b],
                    qry_mega[:, b],
                    cst["wqb"]
                    .rearrange("p (u d) -> p u d", u=1)
                    .to_broadcast([128, 2, 128]),
                )
                nc.vector.tensor_reduce(
                    out=sq_col[:, b], in_=sq_tmp[:, b], axis=X, op=add
                )

                # ---- t(b) on DVE
                g_all = gwork.tile([128, 8, 128], fp32, tag="g_all")
                nc.vector.tensor_mul(
                    g_all.rearrange("p (r h) d -> p r h d", h=2),
                    ctx_mega[:, b].rearrange("p (r h) d -> p r h d", h=2),
                    qwcT[:, b]
                    .rearrange("p (u h) d -> p u h d", u=1)
                    .to_broadcast([128, 4, 2, 128]),
                )
                nc.vector.tensor_reduce(
                    out=t_all[:, b], in_=g_all, axis=X, op=add
                )
                # cs(b) on the PE (accumulating matmuls over k)
                cs_ps = ps_cs.tile([4, 128], fp32, tag="cs")
                for k in range(8):
                    nc.tensor.matmul(
                        cs_ps, cst["rsel"], ctx_mega[:, b, k, :],
                        start=(k == 0), stop=(k == 7),
                    )
                nc.scalar.copy(out=cs[:, b, :], in_=cs_ps)

                # ---- softmax(b)
                nc.vector.tensor_add(
                    t_all[:, s, :].rearrange("p b (r h) -> p b r h", h=2),
                    t_all[:, s, :].rearrange("p b (r h) -> p b r h", h=2),
                    sq_col[:, s, :]
                    .rearrange("p b (u h) -> p b u h", u=1)
                    .to_broadcast([128, 1, 4, 2]),
                )
                nc.scalar.activation(
                    out=eT[:, s, :], in_=t_all[:, s, :], func=Exp
                )
                nc.vector.tensor_add(
                    epair[:, s, :], eT_v[:, s, :, 0], eT_v[:, s, :, 1]
                )
                nc.vector.tensor_add(u4[:, s, :], eT[:, s, 0:4], eT[:, s, 4:8])
                nc.vector.tensor_add(
                    uT[:, s, :], u4[:, s, 0:2], u4[:, s, 2:4]
                )
                nc.vector.reciprocal(out=recu[:, s, :], in_=uT[:, s, :])
                nc.vector.tensor_mul(
                    sqT[:, s, :].rearrange("p b (r h) -> p b r h", h=2),
                    eT_v[:, s, :, :],
                    recu[:, s, :]
                    .rearrange("p b (u h) -> p b u h", u=1)
                    .to_broadcast([128, 1, 4, 2]),
                )

                # ---- projections(b)
                pairs_ps = ps_sm.tile([4, 1], fp32, tag="sm")
                nc.tensor.matmul(
                    pairs_ps, epair[:, b, :], cst["ones1"], start=True, stop=True
                )
                rec4 = pb.tile([4, 1], fp32, tag="rec4")
                nc.vector.reciprocal(out=rec4, in_=pairs_ps)

                sm4u_ps = ps_mm.tile([4, 4], fp32, tag="mm")
                for h in range(2):
                    nc.tensor.matmul(
                        sm4u_ps, sqT_v[:, b, :, h], eT_v[:, b, :, h],
                        start=(h == 0), stop=(h == 1),
                    )
                sm4u = pb.tile([4, 4], fp32, tag="sm4u")
                nc.vector.tensor_copy(out=sm4u, in_=sm4u_ps)

                c2q_ps = ps_mm.tile([4, 128], fp32, tag="mm")
                for h in range(2):
                    nc.tensor.matmul(
                        c2q_ps, eT_v[:, b, :, h], qry_mega[:, b, h, :],
                        start=(h == 0), stop=(h == 1),
                    )
                q2c_ps = ps_mm.tile([4, 128], fp32, tag="mm")
                nc.tensor.matmul(
                    q2c_ps, sm4u, cs[:, b, :], start=True, stop=True
                )
                cq2 = pb.tile([4, 2, 128], fp32, tag="cq2")
                nc.vector.tensor_mul(
                    cq2[:, 0, :], c2q_ps, rec4.to_broadcast([4, 128])
                )
                nc.vector.tensor_mul(
                    cq2[:, 1, :], q2c_ps, rec4.to_broadcast([4, 128])
                )

                rep_ps = ps_rep.tile([128, 2, 128], fp32, tag="rep")
                nc.tensor.matmul(
                    rep_ps,
                    cst["b4"],
                    cq2.rearrange("r g d -> r (g d)"),
                    start=True,
                    stop=True,
                )
                rep = pb.tile([128, 2, 128], fp32, tag="rep")
                nc.scalar.copy(out=rep, in_=rep_ps)

                # ---- assembly(b): C2Q broadcast (ACT), products (DVE/POOL)
                nc.scalar.copy(
                    out=row[:, :, 128:256],
                    in_=rep[:, 0:1, :].to_broadcast([128, 8, 128]),
                )
                p_eng = nc.vector if b % 2 == 0 else nc.gpsimd
                p_eng.tensor_mul(
                    row[:, :, 256:512].rearrange("p k (g d) -> p k g d", g=2),
                    ctx_mega[:, b]
                    .rearrange("p k (u d) -> p k u d", u=1)
                    .to_broadcast([128, 8, 2, 128]),
                    rep[:, :, :]
                    .rearrange("p (u g) d -> p u g d", u=1)
                    .to_broadcast([128, 8, 2, 128]),
                )
                nc.sync.dma_start(
                    out=out_d[b].rearrange("(k p) c -> p k c", p=128),
                    in_=row,
                )
    nc.compile()
    return nc


def _get_program():
    global _prog
    if _prog is None:
        _prog = _build_program()
    return _prog


def _make_const_inputs(w):
    w = np.ascontiguousarray(w, dtype=np.float32)
    w_q, w_c, w_m = w[:D, 0], w[D : 2 * D, 0], w[2 * D :, 0]
    p = np.arange(128)
    vals = {
        "wmb": np.broadcast_to(w_m[None, :], (128, 128)),
        "wcb": np.broadcast_to(w_c[None, :], (128, 128)),
        "wqb": np.broadcast_to(w_q[None, :], (128, 128)),
        "rsel": (p[:, None] % 4 == np.arange(4)[None, :]).astype(np.float32)
        / 256.0,
        "b4": (np.arange(4)[:, None] == p[None, :] % 4).astype(np.float32),
        "ones1": np.ones((128, 1), dtype=np.float32),
    }
    packed = np.zeros((128, _CST_W), dtype=np.float32)
    for n, (parts, c0, cl) in _CST_COLS.items():
        packed[:parts, c0 : c0 + cl] = vals[n]
    return {"cstp": packed}


def _run(context, query, w, trace=False):
    from concourse.bass_utils import run_bass_kernel_spmd

    nc = _get_program()
    context = np.ascontiguousarray(context, dtype=np.float32)
    query = np.ascontiguousarray(query, dtype=np.float32)
    consts = _make_const_inputs(w)

    in_maps = []
    for c in range(NCORES):
        m = {
            "ctx": context[c * BPC : (c + 1) * BPC],
            "qry": query[c * BPC : (c + 1) * BPC],
        }
        m.update(consts)
        in_maps.append(m)

    res = run_bass_kernel_spmd(
        nc, in_maps, core_ids=list(range(NCORES)), trace=trace
    )
    out = np.concatenate([res.results[c]["out"] for c in range(NCORES)], axis=0)
    return out, res


def kernel(context, query, c_mask, q_mask, w):
    out, _ = _run(context, query, w, trace=False)
    return out


# revision 3
# speedup vs baseline: 1.1967x; 1.0185x over previous
"""Trainium2 Bass kernel for nn_ContextQueryAttentionLayer (v8).

Math: idx collapses, S[b,i,:] = t[b, i%4, :]; softmaxes/c2q/sm/q2c are
rank-4 per batch; kernel is HBM-bound (10.5 MB/core, ~30 us roofline).

v8: fully per-batch pipeline, including the loads.  Key lesson from the
v5-v7 traces: engines execute their instruction queues IN ORDER, so any
op emitted early but gated late (e.g. a later batch's ctx->row copy on
ACT) stalls every later op on that engine; and the scheduler orders each
engine's queue by simulated readiness, so batch 0's softmax must be
READY (ACT exp not queue-blocked) when DVE frees or later batches' big
t-ops get packed first.  v8 therefore interleaves qry(b)/ctx(b) loads
(simpler APs also cut HWDGE descriptor-generation latency at the head)
and emits each batch's FULL chain together:
  loads(b) -> qwc(b) -> t(b) -> softmax(b) -> projections(b) ->
  row assembly(b) -> one 2MB full-row DMA (2KB descriptors).
GpSimd takes s_q and the odd batches' product; ACT takes the raw-ctx
and C2Q-broadcast row columns; the PE accumulates cs.

Sharding: data-parallel over batch, 4 batches per core on 8 cores.
On-core layout: ctx rows n=128k+p -> partition p (n%4 == p%4); query
rows j=128h+p -> partition p.
"""

import numpy as np

B, N, M, D = 32, 1024, 256, 128
NCORES = 8
BPC = B // NCORES  # batches per core

_prog = None

# packed constant layout: name -> (partitions, col_start, col_len)
_CST_COLS = {
    "wmb": (128, 0, 128),
    "wcb": (128, 128, 128),
    "wqb": (128, 256, 128),
    "b4": (4, 384, 128),
    "rsel": (128, 512, 4),
    "ones1": (128, 516, 1),
}
_CST_W = 517


def _build_program():
    import concourse.bacc as bacc
    import concourse.mybir as mybir
    from concourse.tile import TileContext

    fp32 = mybir.dt.float32
    nc = bacc.Bacc("TRN2", target_bir_lowering=False, name="cqattn8")

    ctx_d = nc.dram_tensor("ctx", [BPC, N, D], fp32, kind="ExternalInput")
    qry_d = nc.dram_tensor("qry", [BPC, M, D], fp32, kind="ExternalInput")
    cstp_d = nc.dram_tensor("cstp", [128, _CST_W], fp32, kind="ExternalInput")
    out_d = nc.dram_tensor("out", [BPC, N, 4 * D], fp32, kind="ExternalOutput")

    Exp = mybir.ActivationFunctionType.Exp
    add = mybir.AluOpType.add
    X = mybir.AxisListType.X

    with TileContext(nc) as tc:
        with (
            tc.tile_pool(name="consts", bufs=1) as consts,
            tc.tile_pool(name="io", bufs=1) as io,
            tc.tile_pool(name="one", bufs=1) as one,
            tc.tile_pool(name="gwork", bufs=2) as gwork,
            tc.tile_pool(name="pb", bufs=2) as pb,
            tc.tile_pool(name="outp", bufs=4) as outp,
            tc.tile_pool(name="ps_sm", bufs=2, space="PSUM") as ps_sm,
            tc.tile_pool(name="ps_cs", bufs=2, space="PSUM") as ps_cs,
            tc.tile_pool(name="ps_mm", bufs=2, space="PSUM") as ps_mm,
            tc.tile_pool(name="ps_rep", bufs=2, space="PSUM") as ps_rep,
        ):
            # consts first on the scalar ring (simple AP, lands ~2.5us)
            cstp = consts.tile([128, _CST_W], fp32, tag="cstp", name="cstp")
            nc.scalar.dma_start(out=cstp, in_=cstp_d[...])
            cst = {
                n: cstp[:p, c0 : c0 + cl] for n, (p, c0, cl) in _CST_COLS.items()
            }

            qry_mega = io.tile([128, BPC, 2, 128], fp32, tag="qry", name="qry_mega")
            ctx_mega = io.tile([128, BPC, 8, 128], fp32, tag="ctx", name="ctx_mega")

            # shared softmax-domain tiles (written per batch, disjoint slices)
            qwcT = one.tile([128, BPC, 2, 128], fp32, tag="qwcT")
            sq_tmp = one.tile([128, BPC, 2, 128], fp32, tag="sq_tmp")
            sq_col = one.tile([128, BPC, 2], fp32, tag="sq_col")
            t_all = one.tile([128, BPC, 8], fp32, tag="t_all")
            cs = one.tile([4, BPC, 128], fp32, tag="cs")
            eT = one.tile([128, BPC, 8], fp32, tag="eT")
            epair = one.tile([128, BPC, 4], fp32, tag="epair")
            u4 = one.tile([128, BPC, 4], fp32, tag="u4")
            uT = one.tile([128, BPC, 2], fp32, tag="uT")
            recu = one.tile([128, BPC, 2], fp32, tag="recu")
            sqT = one.tile([128, BPC, 8], fp32, tag="sqT")
            eT_v = eT[:, :, :].rearrange("p b (r h) -> p b r h", h=2)
            sqT_v = sqT[:, :, :].rearrange("p b (r h) -> p b r h", h=2)

            for b in range(BPC):
                s = slice(b, b + 1)
                # ---- loads for this batch (sync ring, simple APs)
                nc.sync.dma_start(
                    out=qry_mega[:, b],
                    in_=qry_d[b].rearrange("(h p) d -> p h d", p=128),
                )
                nc.sync.dma_start(
                    out=ctx_mega[:, b],
                    in_=ctx_d[b].rearrange("(k p) d -> p k d", p=128),
                )
                row = outp.tile([128, 8, 512], fp32, tag="row")
                nc.scalar.copy(out=row[:, :, 0:128], in_=ctx_mega[:, b])

                # ---- qwc(b) on DVE, s_q(b) on GpSimd(+DVE reduce)
                nc.vector.tensor_mul(
                    qwcT[:, b],
                    qry_mega[:, b],
                    cst["wmb"]
                    .rearrange("p (u d) -> p u d", u=1)
                    .to_broadcast([128, 2, 128]),
                )
                nc.vector.tensor_add(
                    qwcT[:, b],
                    qwcT[:, b],
                    cst["wcb"]
                    .rearrange("p (u d) -> p u d", u=1)
                    .to_broadcast([128, 2, 128]),
                )
                nc.gpsimd.tensor_mul(
                    sq_tmp[:, b],
                    qry_mega[:, b],
                    cst["wqb"]
                    .rearrange("p (u d) -> p u d", u=1)
                    .to_broadcast([128, 2, 128]),
                )
                nc.vector.tensor_reduce(
                    out=sq_col[:, b], in_=sq_tmp[:, b], axis=X, op=add
                )

                # ---- t(b) on DVE
                g_all = gwork.tile([128, 8, 128], fp32, tag="g_all")
                nc.vector.tensor_mul(
                    g_all.rearrange("p (r h) d -> p r h d", h=2),
                    ctx_mega[:, b].rearrange("p (r h) d -> p r h d", h=2),
                    qwcT[:, b]
                    .rearrange("p (u h) d -> p u h d", u=1)
                    .to_broadcast([128, 4, 2, 128]),
                )
                nc.vector.tensor_reduce(
                    out=t_all[:, b], in_=g_all, axis=X, op=add
                )
                # cs(b) on the PE (accumulating matmuls over k)
                cs_ps = ps_cs.tile([4, 128], fp32, tag="cs")
                for k in range(8):
                    nc.tensor.matmul(
                        cs_ps, cst["rsel"], ctx_mega[:, b, k, :],
                        start=(k == 0), stop=(k == 7),
                    )
                nc.scalar.copy(out=cs[:, b, :], in_=cs_ps)

                # ---- softmax(b)
                nc.vector.tensor_add(
                    t_all[:, s, :].rearrange("p b (r h) -> p b r h", h=2),
                    t_all[:, s, :].rearrange("p b (r h) -> p b r h", h=2),
                    sq_col[:, s, :]
                    .rearrange("p b (u h) -> p b u h", u=1)
                    .to_broadcast([128, 1, 4, 2]),
                )
                nc.scalar.activation(
                    out=eT[:, s, :], in_=t_all[:, s, :], func=Exp
                )
                nc.vector.tensor_add(
                    epair[:, s, :], eT_v[:, s, :, 0], eT_v[:, s, :, 1]
                )
                nc.vector.tensor_add(u4[:, s, :], eT[:, s, 0:4], eT[:, s, 4:8])
                nc.vector.tensor_add(
                    uT[:, s, :], u4[:, s, 0:2], u4[:, s, 2:4]
                )
                nc.vector.reciprocal(out=recu[:, s, :], in_=uT[:, s, :])
                nc.vector.tensor_mul(
                    sqT[:, s, :].rearrange("p b (r h) -> p b r h", h=2),
                    eT_v[:, s, :, :],
                    recu[:, s, :]
                    .rearrange("p b (u h) -> p b u h", u=1)
                    .to_broadcast([128, 1, 4, 2]),
                )

                # ---- projections(b)
                pairs_ps = ps_sm.tile([4, 1], fp32, tag="sm")
                nc.tensor.matmul(
                    pairs_ps, epair[:, b, :], cst["ones1"], start=True, stop=True
                )
                rec4 = pb.tile([4, 1], fp32, tag="rec4")
                nc.vector.reciprocal(out=rec4, in_=pairs_ps)

                sm4u_ps = ps_mm.tile([4, 4], fp32, tag="mm")
                for h in range(2):
                    nc.tensor.matmul(
                        sm4u_ps, sqT_v[:, b, :, h], eT_v[:, b, :, h],
                        start=(h == 0), stop=(h == 1),
                    )
                sm4u = pb.tile([4, 4], fp32, tag="sm4u")
                nc.vector.tensor_copy(out=sm4u, in_=sm4u_ps)

                c2q_ps = ps_mm.tile([4, 128], fp32, tag="mm")
                for h in range(2):
                    nc.tensor.matmul(
                        c2q_ps, eT_v[:, b, :, h], qry_mega[:, b, h, :],
                        start=(h == 0), stop=(h == 1),
                    )
                q2c_ps = ps_mm.tile([4, 128], fp32, tag="mm")
                nc.tensor.matmul(
                    q2c_ps, sm4u, cs[:, b, :], start=True, stop=True
                )
                cq2 = pb.tile([4, 2, 128], fp32, tag="cq2")
                nc.vector.tensor_mul(
                    cq2[:, 0, :], c2q_ps, rec4.to_broadcast([4, 128])
                )
                nc.vector.tensor_mul(
                    cq2[:, 1, :], q2c_ps, rec4.to_broadcast([4, 128])
                )

                rep_ps = ps_rep.tile([128, 2, 128], fp32, tag="rep")
                nc.tensor.matmul(
                    rep_ps,
                    cst["b4"],
                    cq2.rearrange("r g d -> r (g d)"),
                    start=True,
                    stop=True,
                )
                rep = pb.tile([128, 2, 128], fp32, tag="rep")
                nc.scalar.copy(out=rep, in_=rep_ps)

                # ---- assembly(b): C2Q broadcast (ACT), products (DVE/POOL)
                nc.scalar.copy(
                    out=row[:, :, 128:256],
                    in_=rep[:, 0:1, :].to_broadcast([128, 8, 128]),
                )
                p_eng = nc.vector if b % 2 == 0 else nc.gpsimd
                p_eng.tensor_mul(
                    row[:, :, 256:512].rearrange("p k (g d) -> p k g d", g=2),
                    ctx_mega[:, b]
                    .rearrange("p k (u d) -> p k u d", u=1)
                    .to_broadcast([128, 8, 2, 128]),
                    rep[:, :, :]
                    .rearrange("p (u g) d -> p u g d", u=1)
                    .to_broadcast([128, 8, 2, 128]),
                )
                nc.sync.dma_start(
                    out=out_d[b].rearrange("(k p) c -> p k c", p=128),
                    in_=row,
                )
    nc.compile()
    return nc


def _get_program():
    global _prog
    if _prog is None:
        _prog = _build_program()
    return _prog


def _make_const_inputs(w):
    w = np.ascontiguousarray(w, dtype=np.float32)
    w_q, w_c, w_m = w[:D, 0], w[D : 2 * D, 0], w[2 * D :, 0]
    p = np.arange(128)
    vals = {
        "wmb": np.broadcast_to(w_m[None, :], (128, 128)),
        "wcb": np.broadcast_to(w_c[None, :], (128, 128)),
        "wqb": np.broadcast_to(w_q[None, :], (128, 128)),
        "rsel": (p[:, None] % 4 == np.arange(4)[None, :]).astype(np.float32)
        / 256.0,
        "b4": (np.arange(4)[:, None] == p[None, :] % 4).astype(np.float32),
        "ones1": np.ones((128, 1), dtype=np.float32),
    }
    packed = np.zeros((128, _CST_W), dtype=np.float32)
    for n, (parts, c0, cl) in _CST_COLS.items():
        packed[:parts, c0 : c0 + cl] = vals[n]
    return {"cstp": packed}


def _run(context, query, w, trace=False):
    from concourse.bass_utils import run_bass_kernel_spmd

    nc = _get_program()
    context = np.ascontiguousarray(context, dtype=np.float32)
    query = np.ascontiguousarray(query, dtype=np.float32)
    consts = _make_const_inputs(w)

    in_maps = []
    for c in range(NCORES):
        m = {
            "ctx": context[c * BPC : (c + 1) * BPC],
            "qry": query[c * BPC : (c + 1) * BPC],
        }
        m.update(consts)
        in_maps.append(m)

    res = run_bass_kernel_spmd(
        nc, in_maps, core_ids=list(range(NCORES)), trace=trace
    )
    out = np.concatenate([res.results[c]["out"] for c in range(NCORES)], axis=0)
    return out, res


def kernel(context, query, c_mask, q_mask, w):
    out, _ = _run(context, query, w, trace=False)
    return out


# revision 4
# speedup vs baseline: 1.2356x; 1.0325x over previous
"""Trainium2 Bass kernel for nn_ContextQueryAttentionLayer (v10).

Math: idx collapses, S[b,i,:] = t[b, i%4, :]; softmaxes/c2q/sm/q2c are
rank-4 per batch; kernel is HBM-bound (10.5 MB/core, ~30 us roofline).

v8: fully per-batch pipeline, including the loads.  Key lesson from the
v5-v7 traces: engines execute their instruction queues IN ORDER, so any
op emitted early but gated late (e.g. a later batch's ctx->row copy on
ACT) stalls every later op on that engine; and the scheduler orders each
engine's queue by simulated readiness, so batch 0's softmax must be
READY (ACT exp not queue-blocked) when DVE frees or later batches' big
t-ops get packed first.  v8 therefore interleaves qry(b)/ctx(b) loads
(simpler APs also cut HWDGE descriptor-generation latency at the head)
and emits each batch's FULL chain together:
  loads(b) -> qwc(b) -> t(b) -> softmax(b) -> projections(b) ->
  row assembly(b) -> one 2MB full-row DMA (2KB descriptors).
GpSimd takes s_q and the odd batches' product; ACT takes the raw-ctx
and C2Q-broadcast row columns; the PE accumulates cs.

Sharding: data-parallel over batch, 4 batches per core on 8 cores.
On-core layout: ctx rows n=128k+p -> partition p (n%4 == p%4); query
rows j=128h+p -> partition p.
"""

import numpy as np

B, N, M, D = 32, 1024, 256, 128
NCORES = 8
BPC = B // NCORES  # batches per core

_prog = None

# packed constant layout: name -> (partitions, col_start, col_len)
_CST_COLS = {
    "wmb": (128, 0, 128),
    "wcb": (128, 128, 128),
    "wqb": (128, 256, 128),
    "b4": (4, 384, 128),
    "rsel": (128, 512, 4),
    "ones1": (128, 516, 1),
}
_CST_W = 517


def _build_program():
    import concourse.bacc as bacc
    import concourse.mybir as mybir
    from concourse.tile import TileContext

    fp32 = mybir.dt.float32
    nc = bacc.Bacc("TRN2", target_bir_lowering=False, name="cqattn10")

    ctx_d = nc.dram_tensor("ctx", [BPC, N, D], fp32, kind="ExternalInput")
    qry_d = nc.dram_tensor("qry", [BPC, M, D], fp32, kind="ExternalInput")
    cstp_d = nc.dram_tensor("cstp", [128, _CST_W], fp32, kind="ExternalInput")
    out_d = nc.dram_tensor("out", [BPC, N, 4 * D], fp32, kind="ExternalOutput")

    Exp = mybir.ActivationFunctionType.Exp
    add = mybir.AluOpType.add
    X = mybir.AxisListType.X

    with TileContext(nc) as tc:
        with (
            tc.tile_pool(name="consts", bufs=1) as consts,
            tc.tile_pool(name="io", bufs=1) as io,
            tc.tile_pool(name="one", bufs=1) as one,
            tc.tile_pool(name="gwork", bufs=2) as gwork,
            tc.tile_pool(name="pb", bufs=2) as pb,
            tc.tile_pool(name="outp", bufs=4) as outp,
            tc.tile_pool(name="ps_sm", bufs=2, space="PSUM") as ps_sm,
            tc.tile_pool(name="ps_cs", bufs=2, space="PSUM") as ps_cs,
            tc.tile_pool(name="ps_mm", bufs=2, space="PSUM") as ps_mm,
            tc.tile_pool(name="ps_rep", bufs=2, space="PSUM") as ps_rep,
        ):
            # consts first on the scalar ring (simple AP, lands ~2.5us)
            cstp = consts.tile([128, _CST_W], fp32, tag="cstp", name="cstp")
            nc.scalar.dma_start(out=cstp, in_=cstp_d[...])
            cst = {
                n: cstp[:p, c0 : c0 + cl] for n, (p, c0, cl) in _CST_COLS.items()
            }

            qry_mega = io.tile([128, BPC, 2, 128], fp32, tag="qry", name="qry_mega")
            ctx_mega = io.tile([128, BPC, 8, 128], fp32, tag="ctx", name="ctx_mega")

            # shared softmax-domain tiles (written per batch, disjoint slices)
            qwcT = one.tile([128, BPC, 2, 128], fp32, tag="qwcT")
            sq_tmp = one.tile([128, BPC, 2, 128], fp32, tag="sq_tmp")
            sq_col = one.tile([128, BPC, 2], fp32, tag="sq_col")
            t_all = one.tile([128, BPC, 8], fp32, tag="t_all")
            cs = one.tile([4, BPC, 128], fp32, tag="cs")
            eT = one.tile([128, BPC, 8], fp32, tag="eT")
            epair = one.tile([128, BPC, 4], fp32, tag="epair")
            u4 = one.tile([128, BPC, 4], fp32, tag="u4")
            uT = one.tile([128, BPC, 2], fp32, tag="uT")
            recu = one.tile([128, BPC, 2], fp32, tag="recu")
            sqT = one.tile([128, BPC, 8], fp32, tag="sqT")
            eT_v = eT[:, :, :].rearrange("p b (r h) -> p b r h", h=2)
            sqT_v = sqT[:, :, :].rearrange("p b (r h) -> p b r h", h=2)

            for b in range(BPC):
                s = slice(b, b + 1)
                # ---- loads for this batch (sync ring, simple APs)
                nc.sync.dma_start(
                    out=qry_mega[:, b],
                    in_=qry_d[b].rearrange("(h p) d -> p h d", p=128),
                )
                nc.sync.dma_start(
                    out=ctx_mega[:, b],
                    in_=ctx_d[b].rearrange("(k p) d -> p k d", p=128),
                )
                # raw-ctx passthrough stream on the scalar ring: no ACT
                # queue time, and it fills the DMA gap before the first
                # assembled stream
                nc.scalar.dma_start(
                    out=out_d[b][:, 0:128].rearrange("(k p) c -> p k c", p=128),
                    in_=ctx_mega[:, b],
                )
                row = outp.tile([128, 8, 384], fp32, tag="row")

                # ---- qwc(b) on DVE, s_q(b) on GpSimd(+DVE reduce)
                nc.vector.tensor_mul(
                    qwcT[:, b],
                    qry_mega[:, b],
                    cst["wmb"]
                    .rearrange("p (u d) -> p u d", u=1)
                    .to_broadcast([128, 2, 128]),
                )
                nc.vector.tensor_add(
                    qwcT[:, b],
                    qwcT[:, b],
                    cst["wcb"]
                    .rearrange("p (u d) -> p u d", u=1)
                    .to_broadcast([128, 2, 128]),
                )
                nc.gpsimd.tensor_mul(
                    sq_tmp[:, b],
                    qry_mega[:, b],
                    cst["wqb"]
                    .rearrange("p (u d) -> p u d", u=1)
                    .to_broadcast([128, 2, 128]),
                )
                nc.vector.tensor_reduce(
                    out=sq_col[:, b], in_=sq_tmp[:, b], axis=X, op=add
                )

                # ---- t(b) on DVE
                g_all = gwork.tile([128, 8, 128], fp32, tag="g_all")
                nc.vector.tensor_mul(
                    g_all.rearrange("p (r h) d -> p r h d", h=2),
                    ctx_mega[:, b].rearrange("p (r h) d -> p r h d", h=2),
                    qwcT[:, b]
                    .rearrange("p (u h) d -> p u h d", u=1)
                    .to_broadcast([128, 4, 2, 128]),
                )
                nc.vector.tensor_reduce(
                    out=t_all[:, b], in_=g_all, axis=X, op=add
                )
                # cs(b) on the PE (accumulating matmuls over k)
                cs_ps = ps_cs.tile([4, 128], fp32, tag="cs")
                for k in range(8):
                    nc.tensor.matmul(
                        cs_ps, cst["rsel"], ctx_mega[:, b, k, :],
                        start=(k == 0), stop=(k == 7),
                    )
                nc.scalar.copy(out=cs[:, b, :], in_=cs_ps)

                # ---- softmax(b)
                nc.vector.tensor_add(
                    t_all[:, s, :].rearrange("p b (r h) -> p b r h", h=2),
                    t_all[:, s, :].rearrange("p b (r h) -> p b r h", h=2),
                    sq_col[:, s, :]
                    .rearrange("p b (u h) -> p b u h", u=1)
                    .to_broadcast([128, 1, 4, 2]),
                )
                nc.scalar.activation(
                    out=eT[:, s, :], in_=t_all[:, s, :], func=Exp
                )
                nc.vector.tensor_add(
                    epair[:, s, :], eT_v[:, s, :, 0], eT_v[:, s, :, 1]
                )
                nc.vector.tensor_add(u4[:, s, :], eT[:, s, 0:4], eT[:, s, 4:8])
                nc.vector.tensor_add(
                    uT[:, s, :], u4[:, s, 0:2], u4[:, s, 2:4]
                )
                nc.vector.reciprocal(out=recu[:, s, :], in_=uT[:, s, :])
                nc.vector.tensor_mul(
                    sqT[:, s, :].rearrange("p b (r h) -> p b r h", h=2),
                    eT_v[:, s, :, :],
                    recu[:, s, :]
                    .rearrange("p b (u h) -> p b u h", u=1)
                    .to_broadcast([128, 1, 4, 2]),
                )

                # ---- projections(b)
                pairs_ps = ps_sm.tile([4, 1], fp32, tag="sm")
                nc.tensor.matmul(
                    pairs_ps, epair[:, b, :], cst["ones1"], start=True, stop=True
                )
                rec4 = pb.tile([4, 1], fp32, tag="rec4")
                nc.vector.reciprocal(out=rec4, in_=pairs_ps)

                sm4u_ps = ps_mm.tile([4, 4], fp32, tag="mm")
                for h in range(2):
                    nc.tensor.matmul(
                        sm4u_ps, sqT_v[:, b, :, h], eT_v[:, b, :, h],
                        start=(h == 0), stop=(h == 1),
                    )
                sm4u = pb.tile([4, 4], fp32, tag="sm4u")
                nc.vector.tensor_copy(out=sm4u, in_=sm4u_ps)

                c2q_ps = ps_mm.tile([4, 128], fp32, tag="mm")
                for h in range(2):
                    nc.tensor.matmul(
                        c2q_ps, eT_v[:, b, :, h], qry_mega[:, b, h, :],
                        start=(h == 0), stop=(h == 1),
                    )
                q2c_ps = ps_mm.tile([4, 128], fp32, tag="mm")
                nc.tensor.matmul(
                    q2c_ps, sm4u, cs[:, b, :], start=True, stop=True
                )
                cq2 = pb.tile([4, 2, 128], fp32, tag="cq2")
                nc.vector.tensor_mul(
                    cq2[:, 0, :], c2q_ps, rec4.to_broadcast([4, 128])
                )
                nc.vector.tensor_mul(
                    cq2[:, 1, :], q2c_ps, rec4.to_broadcast([4, 128])
                )

                rep_ps = ps_rep.tile([128, 2, 128], fp32, tag="rep")
                nc.tensor.matmul(
                    rep_ps,
                    cst["b4"],
                    cq2.rearrange("r g d -> r (g d)"),
                    start=True,
                    stop=True,
                )
                rep = pb.tile([128, 2, 128], fp32, tag="rep")
                nc.scalar.copy(out=rep, in_=rep_ps)

                # ---- assembly(b): C2Q broadcast (ACT), products (DVE/POOL)
                nc.scalar.copy(
                    out=row[:, :, 0:128],
                    in_=rep[:, 0:1, :].to_broadcast([128, 8, 128]),
                )
                p_eng = nc.vector if b % 2 == 0 else nc.gpsimd
                p_eng.tensor_mul(
                    row[:, :, 128:384].rearrange("p k (g d) -> p k g d", g=2),
                    ctx_mega[:, b]
                    .rearrange("p k (u d) -> p k u d", u=1)
                    .to_broadcast([128, 8, 2, 128]),
                    rep[:, :, :]
                    .rearrange("p (u g) d -> p u g d", u=1)
                    .to_broadcast([128, 8, 2, 128]),
                )
                nc.sync.dma_start(
                    out=out_d[b][:, 128:512].rearrange("(k p) c -> p k c", p=128),
                    in_=row,
                )
    nc.compile()
    return nc


def _get_program():
    global _prog
    if _prog is None:
        _prog = _build_program()
    return _prog


def _make_const_inputs(w):
    w = np.ascontiguousarray(w, dtype=np.float32)
    w_q, w_c, w_m = w[:D, 0], w[D : 2 * D, 0], w[2 * D :, 0]
    p = np.arange(128)
    vals = {
        "wmb": np.broadcast_to(w_m[None, :], (128, 128)),
        "wcb": np.broadcast_to(w_c[None, :], (128, 128)),
        "wqb": np.broadcast_to(w_q[None, :], (128, 128)),
        "rsel": (p[:, None] % 4 == np.arange(4)[None, :]).astype(np.float32)
        / 256.0,
        "b4": (np.arange(4)[:, None] == p[None, :] % 4).astype(np.float32),
        "ones1": np.ones((128, 1), dtype=np.float32),
    }
    packed = np.zeros((128, _CST_W), dtype=np.float32)
    for n, (parts, c0, cl) in _CST_COLS.items():
        packed[:parts, c0 : c0 + cl] = vals[n]
    return {"cstp": packed}


def _run(context, query, w, trace=False):
    from concourse.bass_utils import run_bass_kernel_spmd

    nc = _get_program()
    context = np.ascontiguousarray(context, dtype=np.float32)
    query = np.ascontiguousarray(query, dtype=np.float32)
    consts = _make_const_inputs(w)

    in_maps = []
    for c in range(NCORES):
        m = {
            "ctx": context[c * BPC : (c + 1) * BPC],
            "qry": query[c * BPC : (c + 1) * BPC],
        }
        m.update(consts)
        in_maps.append(m)

    res = run_bass_kernel_spmd(
        nc, in_maps, core_ids=list(range(NCORES)), trace=trace
    )
    out = np.concatenate([res.results[c]["out"] for c in range(NCORES)], axis=0)
    return out, res


def kernel(context, query, c_mask, q_mask, w):
    out, _ = _run(context, query, w, trace=False)
    return out
